# revision 1
# baseline (speedup 1.0000x reference)
"""Trainium2 Bass kernel for CacheShrink MLA attention (8-core SPMD).

Math (matching the reference; dead K/V decompression skipped):
  q = rope(hidden @ Wq) * 1/sqrt(dh)
  c_k, c_v = split(hidden @ Wc)
  per head h (32 heads, GQA onto 4 compressed kv heads):
    S = q_h @ c_k[kv(h)].T  (causal)
    P = exp(S)              (scores are bounded ~[-10, 11], no max needed)
    ctx_h = (P @ c_v[kv(h)]) / rowsum(P)
  out = ctx @ Wo

Sharding: tensor-parallel over heads. Core i owns query heads 4i..4i+3
(all mapping to compressed kv head i//2, so each core computes only its
own 128-dim slice of c_k/c_v from a column slice of Wc). After
attention, bf16 ctx^T shards are AllGather'd (2 MB/rank) and each core
computes a disjoint 512-column block of the output projection, so no
all-reduce is needed. Everything on-chip runs in a transposed layout
(t on the free dim) so every matmul operand is produced in its natural
orientation; the host transposes the final (4096, 2048) result once.

All matmuls are bf16 with f32 PSUM accumulation. The causal mask for
diagonal blocks is folded into the QK accumulation group (a PE
transpose seeds the PSUM bank with -1e30 above the diagonal, then the
QK matmul accumulates on top). The softmax denominator is a
ones-vector matmul over the bf16 probs; its reciprocal is broadcast
across partitions with a rank-1 PE matmul and folded into ctx before
the output projection.
"""

import os
import numpy as np
import ml_dtypes

_SKIP = set(os.environ.get("K_SKIP", "").split(","))

import concourse.bass as bass
import concourse.mybir as mybir
import concourse.tile as tile
from concourse import bacc
from concourse.bass_utils import run_bass_kernel_spmd

BF16 = mybir.dt.bfloat16
F32 = mybir.dt.float32
PSUM = bass.MemorySpace.PSUM

N_CORES = 8
H_PER_CORE = 4      # query heads per core
DH = 128            # head dim
NKO = 32            # k-tiles over the 4096 ctx rows of Wo (32 heads * 128)
TB = 512            # t-block width (one PSUM bank of f32)
NEG = -1.0e30


def build_nc(T=2048, DM=4096, repeat=1, collective=True):
    """Build the single-core SPMD program (same for all 8 cores).

    repeat: int (all phases) or (r_ab, r_c, r_ag, r_e) for timing bisects.
    """
    if isinstance(repeat, int):
        r_ab = r_c = r_ag = r_e = repeat
    else:
        r_ab, r_c, r_ag, r_e = repeat
    NB = T // TB          # 512-wide t blocks
    JB = TB // 128        # 128-wide s tiles per t block
    NK = DM // 128        # k-tiles over d_model
    NK2 = NK // 2
    NKO2 = NKO // 2
    Exp = mybir.ActivationFunctionType.Exp

    nc = bacc.Bacc("TRN2", target_bir_lowering=True, debug=False,
                   num_devices=N_CORES)

    hT = nc.dram_tensor("hT", [DM, T], BF16, kind="ExternalInput")
    wq = nc.dram_tensor("wq", [DM, H_PER_CORE * DH], BF16,
                        kind="ExternalInput")
    wck = nc.dram_tensor("wck", [DM, DH], BF16, kind="ExternalInput")
    wcv = nc.dram_tensor("wcv", [DM, DH], BF16, kind="ExternalInput")
    wo = nc.dram_tensor("wo", [NKO * 128, H_PER_CORE * DH], BF16,
                        kind="ExternalInput")
    cosT = nc.dram_tensor("cosT", [64, T], F32, kind="ExternalInput")
    sinT = nc.dram_tensor("sinT", [64, T], F32, kind="ExternalInput")
    o_t = nc.dram_tensor("o_t", [H_PER_CORE * DH, T], F32,
                         kind="ExternalOutput")

    # internal DRAM for the ctx all-gather
    ctx_loc = nc.dram_tensor("ctx_loc", [H_PER_CORE * DH, T], BF16)
    ctx_all = nc.dram_tensor("ctx_all", [NKO * 128, T], BF16,
                             addr_space="Shared" if collective else "Local")

    hTr = hT.rearrange("(k p) t -> p k t", p=128)
    ctx_all_r = ctx_all.rearrange("(k p) t -> p k t", p=128)
    o_t_r = o_t.rearrange("(m p) t -> p m t", p=128)

    with tile.TileContext(nc) as tc:
        with tc.tile_pool(name="persist", bufs=1) as pp:
            # constants
            identf = pp.tile([128, 128], F32, tag="identf")
            nc.gpsimd.memset(identf[:], 0.0)
            nc.gpsimd.affine_select(
                out=identf[:], in_=identf[:],
                compare_op=mybir.AluOpType.not_equal, fill=1.0,
                base=0, pattern=[[-1, 128]], channel_multiplier=1)
            identb = pp.tile([128, 128], BF16, tag="identb")
            nc.gpsimd.memset(identb[:], 0.0)
            nc.gpsimd.affine_select(
                out=identb[:], in_=identb[:],
                compare_op=mybir.AluOpType.not_equal, fill=1.0,
                base=0, pattern=[[-1, 128]], channel_multiplier=1)
            # maskaddT[t, s] = NEG if s > t else 0; its PE transpose seeds
            # diagonal ST banks with mask[s, t] = NEG above the diagonal.
            maskaddT = pp.tile([128, 128], F32, tag="maskaddT")
            nc.gpsimd.memset(maskaddT[:], 0.0)
            nc.gpsimd.affine_select(
                out=maskaddT[:], in_=maskaddT[:],
                compare_op=mybir.AluOpType.is_ge, fill=NEG,
                base=0, pattern=[[-1, 128]], channel_multiplier=1)
            ones = pp.tile([128, 1], BF16, tag="ones")
            nc.gpsimd.memset(ones[:], 1.0)
            onesrow = pp.tile([1, 128], F32, tag="onesrow")
            nc.gpsimd.memset(onesrow[:], 1.0)

            # weights + rope tables (resident)
            wq_sb = pp.tile([128, NK, H_PER_CORE * DH], BF16, tag="wq")
            nc.sync.dma_start(wq_sb[:], wq.rearrange("(k p) n -> p k n", p=128))
            wck_sb = pp.tile([128, NK, DH], BF16, tag="wck")
            nc.sync.dma_start(wck_sb[:], wck.rearrange("(k p) n -> p k n", p=128))
            wcv_sb = pp.tile([128, NK, DH], BF16, tag="wcv")
            nc.sync.dma_start(wcv_sb[:], wcv.rearrange("(k p) n -> p k n", p=128))
            wo_sb = pp.tile([128, NKO, H_PER_CORE * DH], BF16, tag="wo")
            nc.sync.dma_start(wo_sb[:], wo.rearrange("(k p) n -> p k n", p=128))
            cos_sb = pp.tile([64, T], F32, tag="cos")
            nc.sync.dma_start(cos_sb[:], cosT[:])
            sin_sb = pp.tile([64, T], F32, tag="sin")
            nc.sync.dma_start(sin_sb[:], sinT[:])

            # per-core activations (persist across phases)
            qrT = [pp.tile([128, T], BF16, tag=f"qrT{h}", name=f"qrT{h}")
                   for h in range(H_PER_CORE)]
            ckT_sb = pp.tile([128, T], BF16, tag="ckT")
            cv_sb = pp.tile([128, T], BF16, tag="cv")  # T/128 tiles [s128, d128]

            for _ in range(r_ab):
                # ---- Phase AB: q/ck/cv projections (+rope) ----
                with (
                    tc.tile_pool(name="slab", bufs=3) as slabp,
                    tc.tile_pool(name="abw", bufs=4) as abw,
                    tc.tile_pool(name="qps", bufs=4, space=PSUM) as qpsp,
                    tc.tile_pool(name="kvps", bufs=2, space=PSUM) as kvpsp,
                    tc.tile_pool(name="trps", bufs=1, space=PSUM) as trpsp,
                ):
                    for b in range(NB):
                        bc = slice(b * TB, (b + 1) * TB)
                        qps = [qpsp.tile([128, TB], F32, tag="q", name=f"qps{b}_{hh}")
                               for hh in range(H_PER_CORE)]
                        ckp = kvpsp.tile([128, TB], F32, tag="ckv")
                        cvp = kvpsp.tile([128, TB], F32, tag="ckv")
                        for g in range(2):
                            slab = slabp.tile([128, NK2, TB], BF16, tag="slab")
                            nc.sync.dma_start(
                                slab[:], hTr[:, g * NK2:(g + 1) * NK2, bc])
                            for k2 in range(NK2):
                                k = g * NK2 + k2
                                st = (k == 0)
                                sp = (k == NK - 1)
                                for h in range(H_PER_CORE):
                                    nc.tensor.matmul(
                                        qps[h][:],
                                        wq_sb[:, k, h * DH:(h + 1) * DH],
                                        slab[:, k2, :], start=st, stop=sp)
                                nc.tensor.matmul(ckp[:], wck_sb[:, k, :],
                                                 slab[:, k2, :], start=st, stop=sp)
                                nc.tensor.matmul(cvp[:], wcv_sb[:, k, :],
                                                 slab[:, k2, :], start=st, stop=sp)
                        # rope drain: qrT = rope(q) (scale folded into tables)
                        for h in range(H_PER_CORE):
                            t1 = abw.tile([64, TB], F32, tag="t1")
                            t2 = abw.tile([64, TB], F32, tag="t2")
                            nc.vector.tensor_mul(t1[:], qps[h][0:64, :], cos_sb[:, bc])
                            nc.vector.tensor_mul(t2[:], qps[h][64:128, :], sin_sb[:, bc])
                            nc.vector.tensor_sub(qrT[h][0:64, bc], t1[:], t2[:])
                            t3 = abw.tile([64, TB], F32, tag="t1")
                            t4 = abw.tile([64, TB], F32, tag="t2")
                            nc.vector.tensor_mul(t3[:], qps[h][64:128, :], cos_sb[:, bc])
                            nc.vector.tensor_mul(t4[:], qps[h][0:64, :], sin_sb[:, bc])
                            nc.vector.tensor_add(qrT[h][64:128, bc], t3[:], t4[:])
                        nc.vector.tensor_copy(ckT_sb[:, bc], ckp[:])
                        cvt = abw.tile([128, TB], BF16, tag="cvt")
                        nc.vector.tensor_copy(cvt[:], cvp[:])
                        trp = trpsp.tile([128, JB, 128], BF16, tag="tr")
                        for jl in range(JB):
                            nc.tensor.transpose(
                                trp[:, jl, :], cvt[:, jl * 128:(jl + 1) * 128],
                                identb[:])
                        nc.vector.tensor_copy(cv_sb[:, bc], trp[:])

            for _ in range(r_c):
                # ---- Phase C: attention (transposed layout) ----
                # Per (head, t-block) unit: pass 1 computes all QK tiles and
                # exps them into SBUF probs; pass 2 runs the PV/denominator
                # accumulation. Units are software-pipelined one deep so the
                # ACT exps of unit u+1 overlap the PE PV pass of unit u.
                with (
                    tc.tile_pool(name="cwork", bufs=3) as cw,
                    tc.tile_pool(name="probs", bufs=36) as prp,
                    tc.tile_pool(name="stps", bufs=3, space=PSUM) as stp,
                    tc.tile_pool(name="ctxps", bufs=2, space=PSUM) as ctxp,
                    tc.tile_pool(name="denps", bufs=2, space=PSUM) as denp,
                    tc.tile_pool(name="bcps", bufs=1, space=PSUM) as bcp,
                ):
                    units = [(h, b) for h in range(H_PER_CORE)
                             for b in range(NB)]

                    def qk_pass(h, b):
                        nj = JB * (b + 1)
                        probs = []
                        for j in range(nj):
                            c = j - JB * b          # >=0 on diagonal tiles
                            lo = 128 * max(c, 0)
                            stps = stp.tile([128, TB], F32, tag="st",
                                            name=f"st{h}_{b}_{j}")
                            if "qk" in _SKIP:
                                pr = prp.tile([128, TB], BF16, tag="probs",
                                              name=f"pr{h}_{b}_{j}")
                                probs.append((j, lo, pr))
                                continue
                            if c >= 0 and "masktr" not in _SKIP:
                                # seed bank: clears has_written, writes NEG
                                # mask above diagonal in cols [lo:lo+128]
                                nc.tensor.transpose(
                                    stps[:, lo:lo + 128], maskaddT[:],
                                    identf[:])
                                nc.tensor.matmul(
                                    stps[:],
                                    ckT_sb[:, j * 128:(j + 1) * 128],
                                    qrT[h][:, b * TB:(b + 1) * TB],
                                    start=False, stop=True,
                                    skip_group_check=True)
                            else:
                                nc.tensor.matmul(
                                    stps[:],
                                    ckT_sb[:, j * 128:(j + 1) * 128],
                                    qrT[h][:, b * TB:(b + 1) * TB],
                                    start=True, stop=True)
                            pr = prp.tile([128, TB], BF16, tag="probs",
                                          name=f"pr{h}_{b}_{j}")
                            if "exp" not in _SKIP:
                                nc.scalar.activation(pr[:, lo:], stps[:, lo:], Exp)
                            probs.append((j, lo, pr))
                        return probs

                    def pv_pass(h, b, probs):
                        nj = JB * (b + 1)
                        ctxps = ctxp.tile([128, TB], F32, tag="ctx",
                                          name=f"ctx{h}_{b}")
                        denps = denp.tile([1, TB], F32, tag="den",
                                          name=f"den{h}_{b}")
                        for (j, lo, pr) in probs:
                            if "pv" not in _SKIP:
                                nc.tensor.matmul(
                                    ctxps[:, lo:], cv_sb[:, j * 128:(j + 1) * 128],
                                    pr[:, lo:],
                                    start=(j == 0), stop=(j == nj - 1))
                            if "den" not in _SKIP:
                                nc.tensor.matmul(
                                    denps[:, lo:], ones[:], pr[:, lo:],
                                    start=(j == 0), stop=(j == nj - 1))
                        if "tail" in _SKIP:
                            return
                        rec = cw.tile([1, TB], F32, tag="rec")
                        nc.vector.reciprocal(rec[:], denps[:])
                        bc_ps = bcp.tile([128, TB], F32, tag="bc")
                        nc.tensor.matmul(bc_ps[:], onesrow[:], rec[:])
                        bcs = cw.tile([128, TB], F32, tag="bcs")
                        nc.vector.tensor_copy(bcs[:], bc_ps[:])
                        cn = cw.tile([128, TB], BF16, tag="cn")
                        nc.vector.tensor_mul(cn[:], ctxps[:], bcs[:])
                        nc.sync.dma_start(
                            ctx_loc[h * 128:(h + 1) * 128,
                                    b * TB:(b + 1) * TB], cn[:])

                    prev = None
                    for (h, b) in units:
                        probs = qk_pass(h, b)
                        if prev is not None:
                            pv_pass(*prev)
                        prev = (h, b, probs)
                    pv_pass(*prev)

            for _ in range(r_ag):
                # ---- AllGather ctx across the 8 cores ----
                if collective:
                    nc.gpsimd.collective_compute(
                        "AllGather", mybir.AluOpType.bypass,
                        ins=[ctx_loc[:]], outs=[ctx_all[:]],
                        replica_groups=[list(range(N_CORES))])

            for _ in range(r_e):
                # ---- Phase E: output projection (512-col block) ----
                with (
                    tc.tile_pool(name="cslab", bufs=3) as csp,
                    tc.tile_pool(name="ost", bufs=2) as ostp,
                    tc.tile_pool(name="ops", bufs=2, space=PSUM) as opsp,
                ):
                    for b in range(NB):
                        bc = slice(b * TB, (b + 1) * TB)
                        oacc = opsp.tile([128, H_PER_CORE, TB], F32, tag="o")
                        for g in range(2):
                            cslab = csp.tile([128, NKO2, TB], BF16, tag="cs")
                            nc.sync.dma_start(
                                cslab[:], ctx_all_r[:, g * NKO2:(g + 1) * NKO2, bc])
                            for k2 in range(NKO2):
                                k = g * NKO2 + k2
                                for m in range(H_PER_CORE):
                                    nc.tensor.matmul(
                                        oacc[:, m, :],
                                        wo_sb[:, k, m * 128:(m + 1) * 128],
                                        cslab[:, k2, :],
                                        start=(k == 0), stop=(k == NKO - 1))
                        ost = ostp.tile([128, H_PER_CORE, TB], F32, tag="ost")
                        nc.vector.tensor_copy(ost[:], oacc[:])
                        nc.sync.dma_start(o_t_r[:, :, bc], ost[:])

    nc.compile()
    return nc


_CACHE = {}


def _get_nc(T, DM, repeat=1):
    key = (T, DM, repeat)
    if key not in _CACHE:
        _CACHE[key] = build_nc(T, DM, repeat)
    return _CACHE[key]


def make_inputs(positions, hidden_states, Wq, Wc, Wo, T, DM):
    """Shard + prep the full inputs into 8 per-core input maps."""
    bf = ml_dtypes.bfloat16
    d_latent = Wc.shape[1] // 2
    hT = np.ascontiguousarray(hidden_states.T).astype(bf)

    pos = positions.astype(np.float32)
    inv = (1.0 / (10000.0 ** (np.arange(64, dtype=np.float32) * (2.0 / 128.0))))
    freqs = pos[:, None] * inv[None, :]          # (T, 64) f32
    scale = np.float32(1.0 / np.sqrt(128.0))
    cosT = np.ascontiguousarray((np.cos(freqs) * scale).T)  # (64, T)
    sinT = np.ascontiguousarray((np.sin(freqs) * scale).T)

    in_maps = []
    for i in range(N_CORES):
        kv = i // 2
        in_maps.append({
            "hT": hT,
            "wq": np.ascontiguousarray(
                Wq[:, i * H_PER_CORE * DH:(i + 1) * H_PER_CORE * DH]).astype(bf),
            "wck": np.ascontiguousarray(
                Wc[:, kv * DH:(kv + 1) * DH]).astype(bf),
            "wcv": np.ascontiguousarray(
                Wc[:, d_latent + kv * DH:d_latent + (kv + 1) * DH]).astype(bf),
            "wo": np.ascontiguousarray(
                Wo[:, i * H_PER_CORE * DH:(i + 1) * H_PER_CORE * DH]).astype(bf),
            "cosT": cosT,
            "sinT": sinT,
        })
    return in_maps


def kernel(positions, hidden_states, Wq, Wc, Wuk, Wuv, Wo):
    positions = np.asarray(positions)
    hidden_states = np.asarray(hidden_states, dtype=np.float32)
    Wq = np.asarray(Wq, dtype=np.float32)
    Wc = np.asarray(Wc, dtype=np.float32)
    Wo = np.asarray(Wo, dtype=np.float32)
    T, DM = hidden_states.shape

    nc = _get_nc(T, DM)
    in_maps = make_inputs(positions, hidden_states, Wq, Wc, Wo, T, DM)
    res = run_bass_kernel_spmd(nc, in_maps, list(range(N_CORES))).results
    oT = np.concatenate([res[i]["o_t"] for i in range(N_CORES)], axis=0)
    return np.ascontiguousarray(oT.T)

